# revision 50
# baseline (speedup 1.0000x reference)
"""Multi-head causal attention (B=2, T=2048, E=1024, H=16, D=64) on 8 trn2 cores.

Sharding: tensor-parallel over heads — core c owns heads {2c, 2c+1} (a 128-wide
slice of the hidden dim). Each core computes q/k/v projections for its heads
over the full sequence, causal attention, and a partial output projection
(contraction over its 128 rows of Wo). The host sums the 8 bf16 partials + bias.

Per-core device program (SPMD — one NEFF, different weight slices per core):
  projections: QT/KT = (W.T @ xT) in [dim, token] layout; V in natural
    [token, dim|1] layout (the ones column makes P@V emit Z = sum(exp) free).
    Batch-0 projections run up front (fine-grained x DMAs for a fast start);
    batch-1 projection units are deferred into batch-0's attention waves as
    PE filler.
  attention, per (batch, 512-wide tq chunk), one 128-row tk block per wave:
    S^T for both heads into one [128, 2, 512] PSUM tile; exp on ScalarE in a
    single call covering both heads, trimmed to the causally valid column
    range; diagonal-block triangle mask multiplied on VectorE (bf16 2x mode);
    O^T|Z accumulated per head with causality-trimmed ranges.
  normalize: 1/Z (fp16) per head, K=1 ones-matmul broadcast of 1/Z across the
    64 head dims, then VectorE multiply reading both PSUM operands directly.
  output: out[tq, :] = O^T.T @ Wo_slice per 128 rows, staged to bf16 SBUF on
    the (otherwise idle) GpSimd engine, DMA'd as bf16. Output-projection units
    are deferred and popped into later attention waves so the ScalarE-bound
    batch-1 chunks keep the PE busy.

Timing signal is concourse's TimelineSim cost model (no NTFF profiling under
this axon client). Cost-model notes that shaped this schedule: PE drops to
half clock for 3us after any idle gap; matmul cost = out-free-size x
cycles/row (Ldweights free); engine op cost = free-size x cycle (partition
dim is free parallelism); DMA engines are one shared 360GB/s device.
"""

import numpy as np
import ml_dtypes

import concourse.bass as bass
import concourse.tile as tile
from concourse import bacc, mybir
from concourse.bass_utils import run_bass_kernel_spmd
from contextlib import ExitStack
from collections import deque

B, T, E, H, D = 2, 2048, 1024, 16, 64
BT = B * T            # 4096 tokens total
NCORE = 8
KC = E // 128         # contraction chunks for projections = 8
CQ = 512              # tq chunk width
NQB = T // CQ         # tq chunks per batch = 4
NKB = T // 128        # tk blocks per batch = 16

F32 = mybir.dt.float32
BF16 = mybir.dt.bfloat16
FP16 = mybir.dt.float16
AF = mybir.ActivationFunctionType

_cache = {}

# engine routing knobs (tuned against TimelineSim traces). GpSimd (Pool)
# cannot touch PSUM (BIR verifier rejects it), so PSUM->SBUF copies go to
# DVE/Act; Pool gets the SBUF-only triangle masks.
OST_ENGINE = "vector"    # output staging copies
B1_COPY_ENGINE = "vector"  # batch-1 projection psum->sbuf copies
MASK_ENGINE = "pool"     # diagonal-block triangle masks (SBUF-only op; the
                         # one-wave emission skew hides Pool's latency)


def _build():
    nc = bacc.Bacc("TRN2", target_bir_lowering=False, debug=False,
                   num_devices=NCORE)

    xT = nc.dram_tensor("xT", [E, BT], BF16, kind="ExternalInput").ap()
    wq = nc.dram_tensor("wq", [128, E], BF16, kind="ExternalInput").ap()
    wk = nc.dram_tensor("wk", [128, E], BF16, kind="ExternalInput").ap()
    wv = nc.dram_tensor("wv", [128, E], BF16, kind="ExternalInput").ap()
    wo = nc.dram_tensor("wo", [128, E], BF16, kind="ExternalInput").ap()
    tri = nc.dram_tensor("tri", [128, 2, 128], BF16, kind="ExternalInput").ap()
    eye = nc.dram_tensor("eye", [128, 128], BF16, kind="ExternalInput").ap()
    out = nc.dram_tensor("out", [BT, E], BF16, kind="ExternalOutput").ap()

    with tile.TileContext(nc) as tc, ExitStack() as ctx:
        pers = ctx.enter_context(tc.tile_pool(name="pers", bufs=1))

        wq_sb = pers.tile([128, KC, 128], BF16, tag="wq")
        wk_sb = pers.tile([128, KC, 128], BF16, tag="wk")
        wv_sb = pers.tile([128, KC, 128], BF16, tag="wv")
        wo_sb = pers.tile([128, E], BF16, tag="wo")
        tri_sb = pers.tile([128, 2, 128], BF16, tag="tri")
        eye_sb = pers.tile([128, 128], BF16, tag="eye")
        qt_sb = pers.tile([128, BT], BF16, tag="qt")    # [dims(2 heads), tok]
        kt_sb = pers.tile([128, BT], BF16, tag="kt")
        # V natural + ones col per head: [tok%128, blk, h, d|1]
        v_sb = pers.tile([128, B * NKB, 2, 65], BF16, tag="v")
        ot_sb = pers.tile([128, BT], BF16, tag="ot")    # attn out, [dims, tok]

        xa_pool = ctx.enter_context(tc.tile_pool(name="xa", bufs=16))
        xb_pool = ctx.enter_context(tc.tile_pool(name="xb", bufs=8))
        # PSUM budget (8 banks): sc ring 2x[128,2,512] = 4, pv 2x[128,512] = 2,
        # aux ring 2x[128,512] = 2 shared by V-units / zb broadcast / outproj.
        sc_pool = ctx.enter_context(tc.tile_pool(name="sc", bufs=2, space="PSUM"))
        pv_pool = ctx.enter_context(tc.tile_pool(name="pv", bufs=2, space="PSUM"))
        aux_pool = ctx.enter_context(tc.tile_pool(name="aux", bufs=2, space="PSUM"))
        pt_pool = ctx.enter_context(tc.tile_pool(name="pt", bufs=32))
        zr_pool = ctx.enter_context(tc.tile_pool(name="zr", bufs=2))
        on_pool = ctx.enter_context(tc.tile_pool(name="on", bufs=2))
        ost_pool = ctx.enter_context(tc.tile_pool(name="ost", bufs=4))

        nc.vector.memset(v_sb[:, :, :, 64:65], 1.0)

        def copy_by(eng, dst, src):
            if eng == "act":
                nc.scalar.copy(dst, src)
            elif eng == "pool":
                nc.gpsimd.tensor_copy(dst, src)
            else:
                nc.vector.tensor_copy(dst, src)

        def qk_unit(w_sb, dst_sb, xts, xoff, t_, eng):
            def emit(ctx=None):
                sct = sc_pool.tile([128, 2, CQ], F32, tag="sc",
                                   name=f"qkps{t_}_{id(w_sb)}")
                ps = sct[:, 0, :]
                for kc in range(KC):
                    nc.tensor.matmul(ps, w_sb[:, kc],
                                     xts[kc][:, xoff:xoff + CQ],
                                     start=(kc == 0), stop=(kc == KC - 1))
                copy_by(eng, dst_sb[:, t_ * CQ:(t_ + 1) * CQ], ps)
            return emit

        def v_unit(xts, xoff, t_, eng):
            def emit(ctx=None):
                v_ps = aux_pool.tile([128, CQ], F32, tag="aux", name=f"vps{t_}")
                for j in range(CQ // 128):
                    jf = xoff + j * 128
                    for kc in range(KC):
                        nc.tensor.matmul(
                            v_ps[:, j * 128:(j + 1) * 128],
                            xts[kc][:, jf:jf + 128],
                            wv_sb[:, kc], start=(kc == 0),
                            stop=(kc == KC - 1))
                b4 = t_ * (CQ // 128)
                copy_by(eng, v_sb[:, b4:b4 + 4, :, 0:64],
                        v_ps[:].rearrange("p (j h v) -> p j h v", j=4, h=2))
            return emit

        # ---- weight + batch-0 x DMAs, ordered for a fast start ----
        xa = []
        for pair in range(2):
            xa.append([xa_pool.tile([128, 2 * CQ], BF16, tag="xa",
                                    name=f"xa{pair}_{kc}")
                       for kc in range(KC)])
        # split the first weight chunk + first x chunk so the very first
        # matmul's inputs arrive ahead of the bulk transfers
        wq_r = wq.rearrange("p (kc d) -> p kc d", kc=KC)
        nc.sync.dma_start(wq_sb[:, 0], wq_r[:, 0])
        nc.sync.dma_start(xa[0][0][:, 0:CQ], xT[0:128, 0:CQ])
        nc.sync.dma_start(wq_sb[:, 1:], wq_r[:, 1:])
        nc.sync.dma_start(xa[0][0][:, CQ:], xT[0:128, CQ:2 * CQ])
        nc.sync.dma_start(xa[0][1][:], xT[128:256, 0:2 * CQ])
        nc.sync.dma_start(wk_sb[:], wk.rearrange("p (kc d) -> p kc d", kc=KC))
        nc.sync.dma_start(xa[0][2][:], xT[256:384, 0:2 * CQ])
        nc.sync.dma_start(wv_sb[:], wv.rearrange("p (kc d) -> p kc d", kc=KC))
        for kc in range(3, KC):
            nc.sync.dma_start(xa[0][kc][:],
                              xT[kc * 128:(kc + 1) * 128, 0:2 * CQ])

        # batch-0 projections: braid the pair's two Q and two K chains across
        # four distinct PSUM banks (accumulation groups are bank-scoped, so
        # four concurrent chains in four banks are legal) — each arriving x
        # DMA feeds ~850ns of PE work instead of 213ns, tracking the DMA rate.
        def qk_pair_braided(pair, t0):
            sq0 = sc_pool.tile([128, 2, CQ], F32, tag="sc", name=f"bq{t0}")
            sk0 = sc_pool.tile([128, 2, CQ], F32, tag="sc", name=f"bk{t0}")
            q1 = aux_pool.tile([128, CQ], F32, tag="aux", name=f"bq{t0 + 1}")
            k1 = aux_pool.tile([128, CQ], F32, tag="aux", name=f"bk{t0 + 1}")
            for kc in range(KC):
                st, sp = kc == 0, kc == KC - 1
                x = xa[pair][kc]
                nc.tensor.matmul(sq0[:, 0, :], wq_sb[:, kc], x[:, 0:CQ],
                                 start=st, stop=sp)
                nc.tensor.matmul(sk0[:, 0, :], wk_sb[:, kc], x[:, 0:CQ],
                                 start=st, stop=sp)
                nc.tensor.matmul(q1[:], wq_sb[:, kc], x[:, CQ:],
                                 start=st, stop=sp)
                nc.tensor.matmul(k1[:], wk_sb[:, kc], x[:, CQ:],
                                 start=st, stop=sp)
            nc.scalar.copy(qt_sb[:, t0 * CQ:(t0 + 1) * CQ], sq0[:, 0, :])
            nc.scalar.copy(kt_sb[:, t0 * CQ:(t0 + 1) * CQ], sk0[:, 0, :])
            nc.scalar.copy(qt_sb[:, (t0 + 1) * CQ:(t0 + 2) * CQ], q1[:])
            nc.scalar.copy(kt_sb[:, (t0 + 1) * CQ:(t0 + 2) * CQ], k1[:])

        qk_pair_braided(0, 0)
        for kc in range(KC):
            nc.sync.dma_start(xa[1][kc][:],
                              xT[kc * 128:(kc + 1) * 128, 2 * CQ:4 * CQ])
        nc.sync.dma_start(tri_sb[:], tri[:])
        nc.sync.dma_start(eye_sb[:], eye[:])
        nc.sync.dma_start(wo_sb[:], wo[:])
        v_unit(xa[0], 0, 0, "act")()
        v_unit(xa[0], CQ, 1, "act")()
        qk_pair_braided(1, 2)
        v_unit(xa[1], 0, 2, "act")()
        v_unit(xa[1], CQ, 3, "act")()

        # ---- batch-1 x DMAs (coarse) + deferred projection units ----
        xb = [xb_pool.tile([128, 4 * CQ], BF16, tag="xb", name=f"xb_{kc}")
              for kc in range(KC)]
        for kc in range(KC):
            nc.sync.dma_start(xb[kc][:],
                              xT[kc * 128:(kc + 1) * 128, 4 * CQ:8 * CQ])

        # two filler queues: batch-1 projection units feed batch-0's
        # attention waves; deferred outproj halves feed batch-1's.
        proj_filler = deque()
        op_filler = deque()
        for t_ in range(4, 8):
            xoff = (t_ - 4) * CQ
            proj_filler.append(
                qk_unit(wq_sb, qt_sb, xb, xoff, t_, B1_COPY_ENGINE))
            proj_filler.append(
                qk_unit(wk_sb, kt_sb, xb, xoff, t_, B1_COPY_ENGINE))
            proj_filler.append(v_unit(xb, xoff, t_, B1_COPY_ENGINE))

        default_ctx = {"ring": aux_pool, "copy_eng": OST_ENGINE}

        def pop(q, n=1, ctx=default_ctx):
            for _ in range(n):
                if q:
                    q.popleft()(ctx)

        def outproj_unit(b, cq, j):
            tqg = b * T + cq * CQ + j * 128
            ost = [None]
            def half(eh):
                def emit(ctx):
                    ring = ctx["ring"]
                    if ring is sc_pool:
                        sct = sc_pool.tile([128, 2, CQ], F32, tag="sc",
                                           name=f"o_{b}_{cq}_{j}_{eh}")
                        o = sct[:, 0, :]
                    else:
                        o = ring.tile([128, CQ], F32, tag="aux",
                                      name=f"o_{b}_{cq}_{j}_{eh}")[:]
                    nc.tensor.matmul(o, ot_sb[:, tqg:tqg + 128],
                                     wo_sb[:, eh * CQ:(eh + 1) * CQ],
                                     start=True, stop=True)
                    if ost[0] is None:
                        ost[0] = ost_pool.tile([128, 2, CQ], BF16, tag="ost",
                                               name=f"ost_{b}_{cq}_{j}")
                    copy_by(ctx["copy_eng"], ost[0][:, eh, :], o)
                    if eh == 1:
                        nc.sync.dma_start(
                            out[tqg:tqg + 128, :],
                            ost[0][:].rearrange("p a b -> p (a b)"))
                return emit
            return [half(0), half(1)]

        # --- deferred, braided P@V ------------------------------------------
        # Transposed P@V: out [tq, 4j, d|z] with V as the 65-wide moving
        # operand — half the PE cost of the O^T orientation, and Z lands
        # per-token on partitions so 1/Z is a per-partition scale.
        # PSUM accumulation groups are bank-scoped in the simulator: a second
        # start=True into a bank corrupts the open group, so the four j-groups
        # sharing a pvt bank run j-outer/kb-inner (sequential per bank), and
        # the whole PV stream of chunk c is braided into chunk c+1's
        # score/exp waves (other banks' matmuls may interleave freely).
        pend = {}  # prev chunk's {items, pvt, b, cq, tb, tq0}

        def pend_emit(n):
            items = pend.get("items")
            if not items:
                return
            b, cq = pend["b"], pend["cq"]
            for _ in range(min(n, len(items))):
                kb, j = items.pop(0)
                for h in range(2):
                    nc.tensor.matmul(
                        pend["pvt"][h][:, j, 0:65],
                        pend["pts"][kb][:, h, j * 128:(j + 1) * 128],
                        v_sb[:, b * NKB + kb, h],
                        start=(kb == 0), stop=(kb == 4 * cq + j))

        def pend_finalize():
            if "items" not in pend:
                return
            pend_emit(1 << 30)
            b, cq, pvt = pend["b"], pend["cq"], pend["pvt"]
            tb, tq0 = pend["tb"], pend["tq0"]
            # normalize: per-token 1/Z (one reciprocal per head covers all
            # four 128-token sub-chunks), then scale-copy to bf16 staging.
            zr4 = zr_pool.tile([128, 2, 4], F32, tag="zr", name=f"zr_{b}_{cq}")
            on_t = on_pool.tile([128, 4, 2, 64], BF16, tag="on",
                                name=f"on_{b}_{cq}")
            for h in range(2):
                nc.vector.reciprocal(zr4[:, h, :], pvt[h][:, :, 64])
            for h in range(2):
                for j in range(4):
                    nc.vector.tensor_scalar_mul(
                        on_t[:, j, h, :], pvt[h][:, j, 0:64],
                        zr4[:, h, j:j + 1])

            # transpose O [tq, d] -> O^T [d, tq] on the PE (identity matmul),
            # deferred as filler so the boundary chain never stalls the PE
            def tr_unit(j):
                tqg = tb + tq0 + j * 128
                def emit(ctx):
                    tp = aux_pool.tile([128, 128], BF16, tag="aux",
                                       name=f"tp_{b}_{cq}_{j}")
                    nc.tensor.transpose(
                        tp[:], on_t[:, j].rearrange("p h v -> p (h v)"),
                        eye_sb[:])
                    nc.vector.tensor_copy(ot_sb[:, tqg:tqg + 128], tp[:])
                return emit
            for j in range(CQ // 128):
                op_filler.append(tr_unit(j))
            for j in range(CQ // 128):
                op_filler.extend(outproj_unit(b, cq, j))
            pend.clear()

        def attention_chunk(b, cq):
            q = proj_filler if b == 0 else op_filler
            tb = b * T
            tq0 = cq * CQ
            nblk = (tq0 + CQ) // 128
            quota = -(-len(pend.get("items", ())) // nblk)  # ceil
            pts = {}
            for kb in range(nblk):
                tk0 = kb * 128
                f0 = max(tk0 - tq0, 0)
                s = tk0 - tq0
                # braided prev-chunk PV + filler go FIRST: the PE executes in
                # order, so ready work must precede scores that may wait on
                # an sc-ring slot (freed by the exp two waves back)
                pend_emit(quota)
                if pend.get("items") == [] and kb < nblk - 1:
                    pend_finalize()  # free the prev chunk's pvt/units early
                if b == 0:
                    if kb % 2 == 1:
                        pop(q)
                    elif not q:
                        pop(op_filler)
                else:
                    pop(q, 2 if 0 <= s < CQ else 1)
                sct = sc_pool.tile([128, 2, CQ], F32, tag="sc",
                                   name=f"sc_{b}_{cq}_{kb}")
                for h in range(2):
                    hs = slice(h * 64, (h + 1) * 64)
                    nc.tensor.matmul(
                        sct[:, h, f0:], kt_sb[hs, tb + tk0:tb + tk0 + 128],
                        qt_sb[hs, tb + tq0 + f0:tb + tq0 + CQ],
                        start=True, stop=True)
                ptt = pt_pool.tile([128, 2, CQ], BF16, tag="pt",
                                   name=f"pt_{b}_{cq}_{kb}")
                pts[kb] = ptt
                nc.scalar.activation(ptt[:, :, f0:], sct[:, :, f0:],
                                     AF.Exp, scale=float(D) ** -0.5)
                if 0 <= s < CQ:  # diagonal: triangle mask, both heads at once
                    m_eng = nc.gpsimd if MASK_ENGINE == "pool" else nc.vector
                    m_eng.tensor_mul(ptt[:, :, s:s + 128],
                                     ptt[:, :, s:s + 128], tri_sb[:])
            pend_finalize()
            # [128, 4, 128] f32 = exactly one PSUM bank per head: the two
            # heads' accumulation groups interleave, so they must not share
            # a bank (groups are bank-scoped); only cols 0:65 of each j used.
            pvt = [pv_pool.tile([128, 4, 128], F32, tag="pv",
                                name=f"pv{h}_{b}_{cq}") for h in range(2)]
            pend.update(
                b=b, cq=cq, tb=tb, tq0=tq0, pvt=pvt, pts=pts,
                items=[(kb, j) for j in range(4)
                       for kb in range(4 * cq + j + 1)])

        for cq in range(NQB):          # batch 0: filler = b1 proj units
            attention_chunk(0, cq)
        for cq in range(NQB - 1, -1, -1):  # batch 1 descending: the final
            attention_chunk(1, cq)         # chunk's PV stream is smallest
        # flush: braid the last chunk's PV stream with the remaining filler,
        # alternating PSUM rings and copy engines to keep it pipelined
        k = 0
        while pend.get("items") or proj_filler or op_filler:
            pend_emit(4)
            if not (proj_filler or op_filler):
                if pend.get("items") == []:
                    pend_finalize()
                continue
            q = proj_filler if proj_filler else op_filler
            q.popleft()({"ring": sc_pool if k % 2 else aux_pool,
                         "copy_eng": "act" if k % 2 else "vector"})
            k += 1
        pend_finalize()  # may enqueue the final chunk's tr/outproj units
        while proj_filler or op_filler:
            q = proj_filler if proj_filler else op_filler
            q.popleft()({"ring": sc_pool if k % 2 else aux_pool,
                         "copy_eng": "act" if k % 2 else "vector"})
            k += 1

    nc.compile()
    return nc


def _host_prep(x, Wq, Wk, Wv, Wo):
    bf = ml_dtypes.bfloat16
    xT = np.ascontiguousarray(
        np.asarray(x, dtype=np.float32).reshape(BT, E).T).astype(bf)

    # tri[p, h, f] = 1 where kept (f >= p), applied to the diagonal 128x128
    # sub-block of P^T (tk on partitions, tq on free), both heads
    p = np.arange(128)[:, None]
    f = np.arange(128)[None, :]
    tri = np.broadcast_to((f >= p).astype(bf)[:, None, :], (128, 2, 128))
    tri = np.ascontiguousarray(tri)
    eye = np.eye(128, dtype=bf)

    def perm(w):
        # [E, 128] -> [128p, kc, 128d] flattened: w[kc*128+p, d] -> out[p, kc, d]
        return np.ascontiguousarray(
            w.reshape(KC, 128, 128).transpose(1, 0, 2).reshape(128, E)).astype(bf)

    Wq = np.asarray(Wq, dtype=np.float32)
    Wk = np.asarray(Wk, dtype=np.float32)
    Wv = np.asarray(Wv, dtype=np.float32)
    Wo = np.asarray(Wo, dtype=np.float32)

    in_maps = []
    for c in range(NCORE):
        sl = slice(c * 128, (c + 1) * 128)
        in_maps.append({
            "xT": xT,
            "wq": perm(Wq[:, sl]),
            "wk": perm(Wk[:, sl]),
            "wv": perm(Wv[:, sl]),
            "wo": np.ascontiguousarray(Wo[sl, :]).astype(bf),
            "tri": tri,
            "eye": eye,
        })
    return in_maps


def kernel(x, Wq, Wk, Wv, Wo, bo, _trace=False, _trace_kwargs=None):
    if "nc" not in _cache:
        _cache["nc"] = _build()
    nc = _cache["nc"]

    in_maps = _host_prep(x, Wq, Wk, Wv, Wo)
    kw = {}
    if _trace:
        kw = dict(trace=True, trace_cores=[0], **(_trace_kwargs or {}))
    res = run_bass_kernel_spmd(nc, in_maps, core_ids=list(range(NCORE)), **kw)
    _cache["last_result"] = res

    total = np.zeros((BT, E), dtype=np.float32)
    for r in res.results:
        total += np.asarray(r["out"], dtype=np.float32)
    total += np.asarray(bo, dtype=np.float32)[None, :]
    return total.reshape(B, T, E)
